# revision 4
# baseline (speedup 1.0000x reference)
"""3x3 median filter (zero-padded) on TRN2, 8 NeuronCores, fp16 internal.

Input  x: (32, 3, 512, 512) float32
Output  : (32, 3, 512, 512) float32; median computed in fp16 (rel err ~5e-4,
harness tolerance 2e-2).

Strategy vs the fp32 baseline (433us, DVE-bound at 15 fp32 min/max ops/elem):
  1. All median min/max ops run in fp16 with dense unit-stride last-dim APs
     -> DVE 2x mode (tensor_tensor supports 2x_1p; measured 0.625 ns/elem vs
     1.25 fp32). GpSimd/ACT cannot run tensor ALU ops (backend engine check),
     so DVE keeps all 15 ops/elem but at twice the rate.
  2. Column-parity-planar layout (odd cols | even cols per image) keeps every
     stage-2 horizontal op dense (stride-2 column views measured 2.5x slower
     per elem). ACT (otherwise idle) does fp32<->fp16 conversion fused with
     the de/re-interleave; GpSimd does pad memsets + store DMA triggers.
  3. Both output-row parities share one padded field tile set (8 slots:
     0-3 odd rows, 4-7 even rows) and zero-stride broadcast APs merge the
     per-parity ops -> ~19 wide DVE instrs per block (~183ns dispatch each).
  4. Software pipelining: ACT stream is ordered [in-convs(k+1), out-convs(k)]
     so next-block conversions happen while the DVE computes block k; without
     this the DVE stalls ~8.5us at every block boundary.

Per-block geometry (4 images x 256-row vertical half, partition p = row pair):
  E rows r0+2p, O rows r0+2p+1, Es rows r0+2p+2, Os rows r0+2p+3 (halves
  overlap 2 rows; all DMAs are full 128-partition transfers).
  stage 1 (vertical, 5 ops/elem): qmn/qmx = min/max(O, Es) shared by both
  row parities; odd rows close sort3 with E, even rows with Os (E and Os
  live in one EOs tile so both parities are one broadcast instr).
  Fields are [Opl(257) | Epl(257)] per slot, zero pads at entries 0 (col -1)
  and 513 (col 512); (odd|even) planar source order makes each field write
  one dense 512-wide AP at [1:513].
  stage 2 (horizontal, 10 ops/elem): U/V/Qn/Qx pair sharing at even columns;
  thirds via a stride-258 h-dim (Opl[m] / Epl[m+1]); med9 = med3(max3(mins),
  med3(meds), min3(maxes)).
"""
import sys

if "/opt/trn_rl_repo" not in sys.path:
    sys.path.insert(0, "/opt/trn_rl_repo")

import numpy as np
import concourse.bacc as bacc
import concourse.mybir as mybir
import concourse.tile as tile
from concourse import bass_utils
from concourse.ap import AP

B, C, H, W = 32, 3, 512, 512
N_CORES = 8
B_PER = B // N_CORES          # 4 batches per core
NIMG = B_PER * C              # 12 images per core
GIMG = 4                      # images per tile group
FW = GIMG * W                 # 2048: free width of full row tiles
HWW = W // 2                  # 256 cols per parity plane
PW = W + 2                    # 514: per-slot padded field width
NS = 2 * GIMG                 # 8 field slots (4 odd-row + 4 even-row)
P = 128

F32 = mybir.dt.float32
F16 = mybir.dt.float16
MIN = mybir.AluOpType.min
MAX = mybir.AluOpType.max

_PROGRAM = None


def _deint_convert(nc, dst16, src32, npart, nimg):
    """fp32 interleaved rows -> fp16 planar (odd cols | even cols per image).
    Two ACT instrs (one per parity) keep all AP strides positive."""
    sv = src32.rearrange("p (i m h) -> p i h m", h=2, m=HWW)[0:npart, 0:nimg]
    dv = dst16.rearrange("p (i h m) -> p i h m", h=2, m=HWW)[0:npart, 0:nimg]
    # dst h=0 is ODD cols (w=2m+1 -> src h=1), dst h=1 is EVEN cols (src h=0)
    nc.scalar.copy(dv[:, :, 0], sv[:, :, 1])
    nc.scalar.copy(dv[:, :, 1], sv[:, :, 0])


def _reint_convert(nc, dst32, src16, npart, nimg):
    """fp16 planar (even out-cols | odd out-cols) -> fp32 interleaved.
    Matched [p][i][h][m] iteration on both sides: one ACT instr."""
    sv = src16.rearrange("p (i h m) -> p i h m", h=2, m=HWW)[0:npart, 0:nimg]
    dv = dst32.rearrange("p (i m h) -> p i h m", h=2, m=HWW)[0:npart, 0:nimg]
    nc.scalar.copy(dv, sv)


class Tiles:
    """Persistent SBUF working set (single-buffered handles; double-buffered
    staging/OUT go through tile pools)."""

    def __init__(self, nc, pm):
        t16 = lambda n, fw: pm.tile([P, fw], F16, tag=n, name=n)
        self.O = t16("O", FW)
        self.Es = t16("Es", FW)
        self.qmn = t16("qmn", FW)
        self.qmx = t16("qmx", FW)
        self.t = t16("t", 2 * FW)          # stage-1 temp; reused as A and TF
        self.Ct = t16("Ct", NS * W)
        self.TBt = t16("TBt", NS * W)
        self.MN1 = t16("MN1", NS * W)
        self.MX1 = t16("MX1", NS * W)
        self.fMN = t16("fMN", NS * PW)
        self.fMD = t16("fMD", NS * PW)
        self.fMX = t16("fMX", NS * PW)
        self.U = t16("U", NS * HWW)
        self.V = t16("V", NS * HWW)
        self.Bt = t16("Bt", NS * W)
        # aliases (disjoint lifetimes within a block, DVE-in-order safe;
        # row tiles O/Es/EOs are NOT aliased so next-block ACT conversions
        # can start right after stage 1 reads them)
        self.Qn = self.qmn                 # qmn dead after fMD write
        self.Qx = self.qmx
        self.A = self.t                    # t dead after fMD write
        self.TF = self.t                   # A dead after MX1

    def memset_pads(self, nc):
        for T in (self.fMN, self.fMD, self.fMX):
            q = T[:].rearrange("p (i q) -> p i q", q=PW)
            nc.gpsimd.memset(q[:, :, 0:PW:PW - 1], 0.0)


def _stage2(nc, ts, npart, nslots, OUT16, nsplit):
    """Horizontal pass on padded field tiles [P, nslots*514] -> OUT16
    [npart, nslots*512] planar (even out-cols | odd out-cols per slot).

    Field slot layout [Opl(257) | Epl(257)]: entry 0 = zero (col -1), entry
    j in 1..256 = odd col 2j-1, entry 257+m (m 0..255) = even col 2m, entry
    513 = zero (col 512)."""
    fv = lambda T: T[:].rearrange("p (i q) -> p i q", q=PW)[0:npart, 0:nslots]
    mn, md, mx = fv(ts.fMN), fv(ts.fMD), fv(ts.fMX)
    hv = lambda T: T[:].rearrange("p (i m) -> p i m", m=HWW)[0:npart, 0:nslots]
    Uv, Vv, Qnv, Qxv = hv(ts.U), hv(ts.V), hv(ts.Qn), hv(ts.Qx)

    # shared pair = original cols (2m, 2m+1) = (Epl[m], Opl[m+1])
    pairE = lambda f: f[:, :, 257:513]
    pairO = lambda f: f[:, :, 1:257]
    nc.vector.tensor_tensor(Uv, pairE(mn), pairO(mn), op=MAX)
    nc.vector.tensor_tensor(Vv, pairE(mx), pairO(mx), op=MIN)
    nc.vector.tensor_tensor(Qnv, pairE(md), pairO(md), op=MIN)
    nc.vector.tensor_tensor(Qxv, pairE(md), pairO(md), op=MAX)

    # thirds for both output-col parities in one AP: h=0 (even out-col w=2m)
    # -> Opl[m] at entry m; h=1 (odd w=2m+1) -> Epl[m+1] at entry 258+m
    def third(f):
        a = f[:]
        return AP(
            a.tensor,
            a.offset,
            [[a.ap[0][0], npart], [PW, nslots], [258, 2], [1, HWW]],
        )

    cat = lambda T: T[:].rearrange("p (i h m) -> p i h m", h=2, m=HWW)[
        0:npart, 0:nslots
    ]
    bc = lambda T: T[:].rearrange("p (i u m) -> p i u m", u=1, m=HWW)[
        0:npart, 0:nslots
    ].broadcast_to((npart, nslots, 2, HWW))

    Acat, Bcat, Ccat, TBcat = cat(ts.A), cat(ts.Bt), cat(ts.Ct), cat(ts.TBt)
    nc.vector.tensor_tensor(Acat, bc(ts.U), third(ts.fMN), op=MAX)
    nc.vector.tensor_tensor(Ccat, bc(ts.V), third(ts.fMX), op=MIN)
    nc.vector.tensor_tensor(TBcat, bc(ts.Qx), third(ts.fMD), op=MIN)
    nc.vector.tensor_tensor(Bcat, bc(ts.Qn), TBcat, op=MAX)

    # final med3(A, B, C)
    MN1cat, MX1cat, TFcat = cat(ts.MN1), cat(ts.MX1), cat(ts.TF)
    nc.vector.tensor_tensor(MN1cat, Acat, Bcat, op=MIN)
    nc.vector.tensor_tensor(MX1cat, Acat, Bcat, op=MAX)
    nc.vector.tensor_tensor(TFcat, MX1cat, Ccat, op=MIN)
    ocat = OUT16.rearrange("p (i h m) -> p i h m", h=2, m=HWW)[0:npart, 0:nslots]
    # chunked final op: lets ACT start out-conversions while the DVE
    # finishes later chunks (nsplit=4 on the last block tightens the drain)
    step = nslots // nsplit
    for j in range(nsplit):
        s = slice(j * step, (j + 1) * step)
        nc.vector.tensor_tensor(ocat[:, s], MN1cat[:, s], TFcat[:, s], op=MAX)


def _issue_loads(nc, pio, xh, g, half):
    r0 = 0 if half == 0 else H - 256 - 2
    i0 = GIMG * g
    img = lambda r_lo: xh[r_lo : min(r_lo + 2 * P, H) : 2, i0 : i0 + GIMG, :]
    E32 = pio.tile([P, FW], F32, tag="E32", name="E32", bufs=2)
    O32 = pio.tile([P, FW], F32, tag="O32", name="O32", bufs=2)
    Es32 = pio.tile([P, FW], F32, tag="Es32", name="Es32", bufs=2)
    Os32 = pio.tile([P, FW], F32, tag="Os32", name="Os32", bufs=2)
    # queue order: the (O, Es) pair feeds qmn/qmx first
    nc.sync.dma_start(Es32[:], img(r0 + 2))
    nc.scalar.dma_start(O32[:], img(r0 + 1))
    nc.sync.dma_start(E32[:], img(r0))
    nc.scalar.dma_start(Os32[:], img(r0 + 3))
    return E32, O32, Es32, Os32


def _issue_in_convs(nc, pm, ts, staging):
    E32, O32, Es32, Os32 = staging
    # EOs double-buffered: next-block conversions never wait on this block's
    # stage-1 reads
    EOs = pm.tile([P, 2 * FW], F16, tag="EOs", name="EOs", bufs=2)
    _deint_convert(nc, ts.O[:], O32[:], P, GIMG)
    _deint_convert(nc, ts.Es[:], Es32[:], P, GIMG)
    _deint_convert(nc, EOs[:, 0:FW], E32[:], P, GIMG)
    _deint_convert(nc, EOs[:, FW : 2 * FW], Os32[:], P, GIMG)
    return EOs


def _issue_compute(nc, pm, ts, EOs, g, half, split_s1=False, last=False):
    nc.vector.tensor_tensor(ts.qmn[:], ts.O[:], ts.Es[:], op=MIN)
    nc.vector.tensor_tensor(ts.qmx[:], ts.O[:], ts.Es[:], op=MAX)

    # dense write view over both parities: [p][rho][i][entries 1..512]
    wf = lambda T: T[:].rearrange("p (r i q) -> p r i q", r=2, q=PW)[
        :, :, :, 1 : PW - 1
    ]
    eos = EOs[:].rearrange("p (r i w) -> p r i w", r=2, w=W)
    qb = lambda T: T[:].rearrange("p (u i w) -> p u i w", u=1, w=W).broadcast_to(
        (P, 2, GIMG, W)
    )
    tv = ts.t[:].rearrange("p (r i w) -> p r i w", r=2, w=W)

    nc.vector.tensor_tensor(wf(ts.fMN), qb(ts.qmn), eos, op=MIN)
    nc.vector.tensor_tensor(wf(ts.fMX), qb(ts.qmx), eos, op=MAX)
    nc.vector.tensor_tensor(tv, qb(ts.qmx), eos, op=MIN)
    nc.vector.tensor_tensor(wf(ts.fMD), qb(ts.qmn), tv, op=MAX)

    OUT16 = pm.tile([P, NS * W], F16, tag="OUT16", name="OUT16", bufs=2)
    _stage2(nc, ts, P, NS, OUT16[:], nsplit=(4 if last else 2))
    return OUT16


def _issue_out(nc, pio, oh, g, half, OUT16, last=False):
    r0 = 0 if half == 0 else H - 256 - 2
    i0 = GIMG * g
    OUT32_o = pio.tile([P, FW], F32, tag="E32", name="OUT32_o", bufs=2)
    OUT32_e = pio.tile([P, FW], F32, tag="O32", name="OUT32_e", bufs=2)
    _reint_convert(nc, OUT32_o[:], OUT16[:, 0:FW], P, GIMG)
    _reint_convert(nc, OUT32_e[:], OUT16[:, FW : 2 * FW], P, GIMG)
    out_img = lambda r_lo: oh[r_lo : min(r_lo + 2 * P, H) : 2, i0 : i0 + GIMG, :]
    if last:
        # HWDGE queues are drained of loads by now; parallel queues shrink
        # the tail (stores elsewhere stay on SWDGE so they never block loads)
        nc.sync.dma_start(out_img(r0 + 1), OUT32_o[:])
        nc.scalar.dma_start(out_img(r0 + 2), OUT32_e[:])
    else:
        nc.gpsimd.dma_start(out_img(r0 + 1), OUT32_o[:])
        nc.gpsimd.dma_start(out_img(r0 + 2), OUT32_e[:])


def _issue_out_last(nc, pio, oh, g, half, OUT16):
    """Last block: quartered conversions + stores (2 images each) on the
    now-empty HWDGE queues, so the drain pipeline empties sooner."""
    r0 = 0 if half == 0 else H - 256 - 2
    i0 = GIMG * g
    OUT32_o = pio.tile([P, FW], F32, tag="E32", name="OUT32_o", bufs=2)
    OUT32_e = pio.tile([P, FW], F32, tag="O32", name="OUT32_e", bufs=2)
    HF = FW // 2
    oimg = lambda r_lo, j0: oh[
        r_lo : min(r_lo + 2 * P, H) : 2, i0 + j0 : i0 + j0 + 2, :
    ]
    for j0, q in ((0, nc.sync), (2, nc.scalar)):
        s = slice(j0 * W, j0 * W + HF)
        _reint_convert(nc, OUT32_o[:][:, s], OUT16[:, s], P, 2)
        q.dma_start(oimg(r0 + 1, j0), OUT32_o[:, s])
    for j0, q in ((0, nc.sync), (2, nc.scalar)):
        s = slice(j0 * W, j0 * W + HF)
        _reint_convert(nc, OUT32_e[:][:, s], OUT16[:, FW + j0 * W : FW + j0 * W + HF], P, 2)
        q.dma_start(oimg(r0 + 2, j0), OUT32_e[:, s])


NE = 2 * NIMG


def _edge_loads(nc, pio, xh):
    """Image rows 0/1/510/511 for all 12 images, on the GpSimd SWDGE queue
    (idle at startup; does not delay the block loads). One DMA per tile:
    partitions 0..11 = rows 0 (resp 1), 12..23 = rows 511 (resp 510)."""
    R032 = pio.tile([NE, W], F32, tag="R032", name="R032")
    R132 = pio.tile([NE, W], F32, tag="R132", name="R132")
    nc.gpsimd.dma_start(R032[:], xh[0 : H : H - 1, :, :])
    nc.gpsimd.dma_start(R132[:], xh[1 : H : H - 3, :, :])
    return R032, R132


def _edge_convs(nc, pm, staging):
    R032, R132 = staging
    R0 = pm.tile([NE, W], F16, tag="R0", name="R0")
    R1 = pm.tile([NE, W], F16, tag="R1", name="R1")
    _deint_convert(nc, R0[:], R032[:], NE, 1)
    _deint_convert(nc, R1[:], R132[:], NE, 1)
    return R0, R1


def _edge_compute(nc, pm, ts, rows):
    """p 0..11 = row 0 of image p (partner row 1); p 12..23 = row 511
    (partner row 510). Reuses the persistent field tiles (slot 0)."""
    R0, R1 = rows
    rmn = pm.tile([NE, W], F16, tag="rmn", name="rmn")
    rmx = pm.tile([NE, W], F16, tag="rmx", name="rmx")
    nc.vector.tensor_tensor(rmn[:], R0[:], R1[:], op=MIN)
    nc.vector.tensor_tensor(rmx[:], R0[:], R1[:], op=MAX)

    w = lambda T: T[:].rearrange("p (i q) -> p i q", q=PW)[0:NE, 0:1, 1 : PW - 1]
    r1 = lambda T: T[:].rearrange("p (i w) -> p i w", i=1)
    # sort3 with the zero pad row: min/max vs 0, med = max(rmn, min(rmx, 0))
    nc.vector.tensor_scalar_min(w(ts.fMN), r1(rmn), 0.0)
    nc.vector.tensor_scalar_max(w(ts.fMX), r1(rmx), 0.0)
    nc.vector.scalar_tensor_tensor(
        w(ts.fMD), r1(rmx), 0.0, r1(rmn), op0=MIN, op1=MAX
    )

    OUT16_0 = pm.tile([NE, W], F16, tag="OUT16_0", name="OUT16_0")
    _stage2(nc, ts, NE, 1, OUT16_0[:], nsplit=1)
    return OUT16_0


def _edge_out(nc, pio, oi, OUT16_0):
    OUT32_0 = pio.tile([NE, W], F32, tag="OUT32_0", name="OUT32_0")
    _reint_convert(nc, OUT32_0[:], OUT16_0[:], NE, 1)
    nc.gpsimd.dma_start(oi[:, 0, :], OUT32_0[0:NIMG, :])
    nc.gpsimd.dma_start(oi[:, H - 1, :], OUT32_0[NIMG:NE, :])


def build_program():
    nc = bacc.Bacc(
        "TRN2", target_bir_lowering=False, debug=False, num_devices=N_CORES
    )
    x_d = nc.dram_tensor("x", [B_PER, C, H, W], F32, kind="ExternalInput").ap()
    o_d = nc.dram_tensor("out", [B_PER, C, H, W], F32, kind="ExternalOutput").ap()
    xh = x_d.rearrange("b c h w -> h (b c) w")
    oh = o_d.rearrange("b c h w -> h (b c) w")
    xi = x_d.rearrange("b c h w -> (b c) h w")
    oi = o_d.rearrange("b c h w -> (b c) h w")

    blocks = [(g, half) for g in range(NIMG // GIMG) for half in range(2)]
    NB = len(blocks)

    with tile.TileContext(nc) as tc:
        with (
            tc.tile_pool(name="io", bufs=1) as pio,
            tc.tile_pool(name="mid", bufs=1) as pm,
        ):
            ts = Tiles(nc, pm)
            ts.memset_pads(nc)
            # startup: block-0 loads first on the sync/scalar queues, edge
            # loads on the tensor queue; edge convs first on ACT (small), so
            # the DVE's edge compute covers the block-0 conversion latency
            # startup: block-0 loads go first on the sync/scalar queues, the
            # (small) edge loads on the SWDGE queue; the DVE's edge compute
            # covers part of the ~20us block-0 load+convert latency (all 8
            # cores load simultaneously, DMA is near its aggregate limit)
            # loads are issued TWO blocks ahead: the dma_start dispatches of
            # block k+1 would otherwise sit behind block k's conversions in
            # the ACT stream, which stall waiting for DMA -- delaying the
            # dispatch (and the transfer) by most of a block at ramp-up
            stg = [None] * NB
            stg[0] = _issue_loads(nc, pio, xh, *blocks[0])
            e_staging = _edge_loads(nc, pio, xh)
            stg[1] = _issue_loads(nc, pio, xh, *blocks[1])
            e_rows = _edge_convs(nc, pm, e_staging)
            EOs = _issue_in_convs(nc, pm, ts, stg[0])
            OUT16_0 = _edge_compute(nc, pm, ts, e_rows)
            _edge_out(nc, pio, oi, OUT16_0)
            # software pipeline: in-convs(k+1) are issued before out-convs(k)
            # so ACT converts ahead of the DVE
            for k in range(NB):
                if k + 2 < NB:
                    stg[k + 2] = _issue_loads(nc, pio, xh, *blocks[k + 2])
                OUT16 = _issue_compute(nc, pm, ts, EOs, *blocks[k],
                                       split_s1=(k <= 1), last=(k == NB - 1))
                if k + 1 < NB:
                    EOs = _issue_in_convs(nc, pm, ts, stg[k + 1])
                if k == NB - 1:
                    _issue_out_last(nc, pio, oh, *blocks[k], OUT16)
                else:
                    _issue_out(nc, pio, oh, *blocks[k], OUT16)
    nc.compile()
    return nc


def _get_program():
    global _PROGRAM
    if _PROGRAM is None:
        _PROGRAM = build_program()
    return _PROGRAM


def kernel(**inputs) -> np.ndarray:
    x = np.ascontiguousarray(np.asarray(inputs["x"], dtype=np.float32))
    assert x.shape == (B, C, H, W), x.shape
    nc = _get_program()
    in_maps = [{"x": x[k * B_PER : (k + 1) * B_PER]} for k in range(N_CORES)]
    res = bass_utils.run_bass_kernel_spmd(nc, in_maps, core_ids=list(range(N_CORES)))
    return np.concatenate([res.results[k]["out"] for k in range(N_CORES)], axis=0)
